# revision 26
# baseline (speedup 1.0000x reference)
"""Conv-QKV self-attention (CSA) Trainium2 Bass kernel, v4.

Reference computation (per batch b):
    k = conv1d(x, K_w, K_b); q = conv1d(x, Q_w, Q_b); v = conv1d(x, V_w, V_b)
    scores = relu(k^T q)                # [L, L], contraction over 64 channels
    out = v @ scores / sqrt(3)          # [64, L], contraction over L
Sharding: 8 cores = 4 batches x 2 l-halves of the score matrix.  Each core
computes a PARTIAL out (contraction over its l-half) for the full m range;
the host sums the two partials per batch.  1/sqrt(3) folded into V weights.

v4 design notes (HW-trace driven, on top of v3):
 - The flash phase is DVE+ACT bound, not PE bound: the relu copy of scores
   out of PSUM (fp32, 1 elem/lane/cycle, no 2x modes for fp32-from-PSUM on
   TRN2) costs ~1.2us per [128,1024] tile and there are 64 such tiles.
   Everything else is scheduled to hide under that ~40us/engine wall.
 - mm2 is COLUMN-PACKED: two 512-col m-tiles accumulate concurrently in the
   two column halves of the PE array (tile_position (0,0)/(0,64) via psum
   partition offsets), so PE flash work drops 42.7us -> ~29us.  The shared
   PSUM bank uses ONE accumulation group: start=True only on the very first
   matmul, stop=True only on the very last (has_written bits are
   per-element; start clears the whole bank).
 - Each flash unit = (l-tile j, m-pair): mm1 row-packed pair (k[j]@h0 ->
   cols mA, k[j]@h64 -> cols mB of one 2-bank psum tile), ONE relu op
   [128,1024] (engine chosen by a 30/34 DVE/ACT balance pattern), mm2
   col-packed pair into po[0:64]/po[64:128].
 - Conv copies merged: the fused K|Q conv writes k to psum rows 0:64 and q
   to rows 64:128; ONE [128,512] bias-copy per group (engine cost is free
   dim only).  q-other convs are col-packed pairs (2 groups concurrently on
   the two PE column halves -> one [128,512] copy per 2 groups).  vT convs
   write 8 tiles into one psum bank -> one [128,512] copy per 8 tiles.
   Row-half duplicates (k on h64, q on h0) are made by sbuf->sbuf DMA, off
   the engines.  mm1-B reads q directly from the conv output rows 64:128
   (no duplication); the other-block q pairs need no dup at all.
 - PSUM: score pool 3x[128,1024] (6 banks) + po pool 2x[128,512] (2 banks)
   = exactly 8; conv/warmup psum cycles through the same two rings.
 - preamble: tiny first x chunk + weights first on the gpsimd queue get the
   first warmup matmul ~1.5us earlier; warmup + heat keep HAM at 8/8.
"""

import numpy as np

FIN, FOUT, KS = 64, 64, 3
B, L = 4, 4096
HALF = L // 2            # per-core l range
NCORES = 8
MT = 512                 # m tile (PSUM bank free dim, fp32)
LT = 128                 # l tile (PE partition dim)
N_LT = HALF // LT        # 16 l-tiles in this core's half
N_G = HALF // MT         # 4 conv groups per 2048-col block
N_P = 4                  # m-pairs (each 2x512 cols)
BLK = HALF + 2           # 2050: x block incl +-1 halo
SQRT_KS = float(np.sqrt(KS))
N_WARM = 4               # N=384 warmup matmuls (HAM ramp + DMA preamble cover)
N_ACT_RELU = 35          # relu tiles on ACT (faster); DVE gets 64 - this

_NC_CACHE = {}


def _relu_on_act(u):
    # spread N_ACT_RELU ACT-relus evenly over the 64 units
    return (u * N_ACT_RELU) // 64 != ((u + 1) * N_ACT_RELU) // 64


def _build_nc():
    from contextlib import ExitStack

    import concourse.tile as tile
    from concourse import bacc, mybir

    f32 = mybir.dt.float32
    bf16 = mybir.dt.bfloat16
    AF = mybir.ActivationFunctionType

    nc = bacc.Bacc("TRN2", target_bir_lowering=False)

    # x as two 2050-col halo blocks [own | other], each with a 1-shifted
    # copy on partitions 64:128 (K-stacked taps 0+1), bf16.
    xd_d = nc.dram_tensor("xd", [128, 2 * BLK], bf16, kind="ExternalInput")
    # fused conv weights [128, 256]:
    #   [:,   0: 64] = [Kt0;Kt1]   [:,  64:128] = [Qt0;Qt1]
    #   [0:64,128:192] = Kt2       [0:64,192:256] = Qt2   (rows 64:128 zero)
    kqw_d = nc.dram_tensor("kqw", [128, 4 * FOUT], bf16, kind="ExternalInput")
    # v weights (1/sqrt(3) folded): [:,0:64] = [Vt0;Vt1], [0:64,64:128] = Vt2
    vw_d = nc.dram_tensor("vw", [128, 2 * FOUT], bf16, kind="ExternalInput")
    # col 0 = [K_b; Q_b]; col 1 = [Q_b; Q_b]
    kqb_d = nc.dram_tensor("kqb", [128, 2], f32, kind="ExternalInput")
    # V bias (1/sqrt(3) folded) tiled 8x: [1, 512]
    vb_d = nc.dram_tensor("vb", [1, 8 * FOUT], f32, kind="ExternalInput")
    out_d = nc.dram_tensor("out", [FOUT, L], f32, kind="ExternalOutput")

    with tile.TileContext(nc) as tc, ExitStack() as ctx:
        consts = ctx.enter_context(tc.tile_pool(name="consts", bufs=1))
        big = ctx.enter_context(tc.tile_pool(name="big", bufs=1))
        pps = ctx.enter_context(tc.tile_pool(name="pps", bufs=3, space="PSUM"))
        ppo = ctx.enter_context(tc.tile_pool(name="ppo", bufs=2, space="PSUM"))
        spool = ctx.enter_context(tc.tile_pool(name="spool", bufs=6))
        opool = ctx.enter_context(tc.tile_pool(name="opool", bufs=2))

        # ---- DMA preamble --------------------------------------------
        xd_sb = consts.tile([128, 2 * BLK], bf16)
        # tiny first chunk gates the warmup burst; then the g0 halo.
        # Four queues in parallel: sync = own x, scalar = kq consts,
        # vector = v consts, gpsimd = other-block x + later dups.
        nc.sync.dma_start(out=xd_sb[:, 0:256], in_=xd_d[:, 0:256])
        nc.sync.dma_start(out=xd_sb[:, 256:514], in_=xd_d[:, 256:514])
        kqw_sb = consts.tile([128, 4 * FOUT], bf16)
        nc.scalar.dma_start(out=kqw_sb, in_=kqw_d[:, :])
        kqb_sb = consts.tile([128, 2], f32)
        nc.scalar.dma_start(out=kqb_sb, in_=kqb_d[:, :])
        vw_sb = consts.tile([128, 2 * FOUT], bf16)
        nc.gpsimd.dma_start(out=vw_sb, in_=vw_d[:, :])
        vb_sb = consts.tile([128, 8 * FOUT], f32)
        nc.gpsimd.dma_start(out=vb_sb, in_=vb_d[:, :].to_broadcast([128, 8 * FOUT]))
        # rest of own block split across sync+scalar queues (conv groups
        # are DMA-paced), other block on gpsimd
        nc.scalar.dma_start(out=xd_sb[:, 514:1026], in_=xd_d[:, 514:1026])
        nc.sync.dma_start(out=xd_sb[:, 1026:1538], in_=xd_d[:, 1026:1538])
        nc.scalar.dma_start(out=xd_sb[:, 1538:2050], in_=xd_d[:, 1538:2050])
        nc.gpsimd.dma_start(out=xd_sb[:, BLK : BLK + 1026], in_=xd_d[:, BLK : BLK + 1026])
        nc.gpsimd.dma_start(
            out=xd_sb[:, BLK + 1026 : 2 * BLK], in_=xd_d[:, BLK + 1026 : 2 * BLK]
        )

        # conv outputs
        kqow = big.tile([128, HALF], bf16)   # rows 0:64 k+Kb, rows 64:128 q+Qb
        khi = big.tile([128, HALF], bf16)    # rows 64:128 = k+Kb (dup)
        qlo = big.tile([128, 2 * MT], bf16)  # rows 0:64: q cols {0:512,1024:1536}
        qoth = big.tile([128, 2, MT], bf16)  # [:,t,:]: rows 0:64 q grp 2t, rows 64:128 grp 2t+1
        vt_sb = big.tile([128, N_LT, FOUT], bf16)

        # ---- warmup burst on the first landed x chunk ----------------
        # HAM needs ~3.4us of dense, real-data PE activity to reach 8/8.
        # A short N=128 burst covers the kqw-DMA wait; the conv matmuls
        # and N=384 heats between conv groups provide the rest, so the
        # clocks ramp ~when the flash loop starts.
        wscr = consts.tile([128, 512], bf16)
        for i in range(8):
            wp = pps.tile([128, 2 * MT], f32, name="wp", tag="ps")
            nc.tensor.matmul(
                wp[:, 0:128], xd_sb[:, 0:128], xd_sb[:, 128:256],
                start=True, stop=True,
            )
            if i == 1:
                nc.vector.tensor_scalar_max(wscr[:, 0:256], xd_sb[:, 0:256], 0.0)
            elif i == 2:
                nc.scalar.activation(wscr[:, 0:256], xd_sb[:, 0:256], AF.Relu)
        for i in range(N_WARM):
            wp = pps.tile([128, 2 * MT], f32, name="wp", tag="ps")
            nc.tensor.matmul(
                wp[:, 0:384], xd_sb[:, 0:128], xd_sb[:, 128:512],
                start=True, stop=True,
            )

        def heat(n):
            for _ in range(n):
                hp = pps.tile([128, 2 * MT], f32, name="hp", tag="ps")
                nc.tensor.matmul(
                    hp[:, 0:384], xd_sb[:, 0:128], xd_sb[:, 128:512],
                    start=True, stop=True,
                )

        # ---- fused K|Q conv over the own block -----------------------
        for g in range(N_G):
            p = pps.tile([128, 2 * MT], f32, name="pkq", tag="ps")
            nc.tensor.matmul(
                p[:, 0:MT], kqw_sb[:, 0:128], xd_sb[:, g * MT : g * MT + MT],
                start=True, stop=False,
            )
            nc.tensor.matmul(
                p[:, 0:MT], kqw_sb[0:FIN, 128:256],
                xd_sb[0:FIN, g * MT + 2 : g * MT + 2 + MT],
                start=False, stop=True,
            )
            gsl = slice(g * MT, (g + 1) * MT)
            if g % 2 == 0:
                nc.scalar.activation(
                    kqow[:, gsl], p[:, 0:MT], AF.Identity, bias=kqb_sb[:, 0:1]
                )
            else:
                nc.vector.tensor_scalar_add(kqow[:, gsl], p[:, 0:MT], kqb_sb[:, 0:1])
            heat(2)
            # khi dup per group so the first flash units unblock early
            nc.sync.dma_start(out=khi[FIN:128, gsl], in_=kqow[0:FIN, gsl])
            if g == 0:
                nc.gpsimd.dma_start(out=qlo[0:FIN, 0:MT], in_=kqow[FIN:128, 0:MT])
            if g == 2:
                nc.gpsimd.dma_start(
                    out=qlo[0:FIN, MT : 2 * MT], in_=kqow[FIN:128, 1024:1536]
                )

        # ---- conv work interleaved into the early flash stream -------
        # vT tiles: 8 per psum bank (one engine copy per bank); q-other
        # col-packed conv pairs.  Emitted between early flash units so PE
        # keeps the engines fed while these fill in; their consumers
        # (mm2 / pairs 2-3) are many units downstream.
        pv_cur = [None]
        pq_cur = [None]

        def vt_block(blk):
            pv = ppo.tile([128, MT], f32, name="pv", tag="po")
            pv_cur[0] = pv
            for i in range(8):
                j = blk * 8 + i
                vsl = slice(i * FOUT, (i + 1) * FOUT)
                nc.tensor.matmul(
                    pv[:, vsl], xd_sb[:, j * LT : j * LT + LT], vw_sb[:, 0:FOUT],
                    start=(i == 0), stop=False,
                )
                nc.tensor.matmul(
                    pv[:, vsl], xd_sb[0:FIN, j * LT + 2 : j * LT + 2 + LT],
                    vw_sb[0:FIN, FOUT:128],
                    start=False, stop=(i == 7),
                )
                if i % 2 == 1:
                    # N=64 streams read as "idle" to the HAM activity
                    # monitor; keep real N=384 bursts in the mix
                    heat(1)
            nc.vector.tensor_add(
                vt_sb[:, blk * 8 : (blk + 1) * 8, :], pv, vb_sb[:, 0:MT]
            )

        def qoth_conv(t):
            pq = pps.tile([128, 2 * MT], f32, name="pq", tag="ps")
            pq_cur[0] = pq
            for tap in range(2):
                for h in range(2):
                    lo = BLK + (2 * t + h) * MT + 2 * tap
                    osl = pq[h * FOUT : (h + 1) * FOUT, 0:MT]
                    if tap == 0:
                        nc.tensor.matmul(
                            osl, kqw_sb[:, 64:128], xd_sb[:, lo : lo + MT],
                            start=True, stop=False, skip_group_check=True,
                        )
                    else:
                        nc.tensor.matmul(
                            osl, kqw_sb[0:FIN, 192:256], xd_sb[0:FIN, lo : lo + MT],
                            start=False, stop=True, skip_group_check=True,
                        )

        def qoth_copy(t):
            pq = pq_cur[0]
            if t == 0:
                nc.scalar.activation(
                    qoth[:, t, :], pq[:, 0:MT], AF.Identity, bias=kqb_sb[:, 1:2]
                )
            else:
                nc.vector.tensor_scalar_add(qoth[:, t, :], pq[:, 0:MT], kqb_sb[:, 1:2])

        INTERLEAVE = {
            1: lambda: vt_block(0),
            3: lambda: vt_block(1),
            6: lambda: qoth_conv(0),
            7: lambda: qoth_copy(0),
            8: lambda: qoth_conv(1),
            9: lambda: qoth_copy(1),
        }

        # ---- flash loop ----------------------------------------------
        # m-pair p covers out columns (pA, pB) in the block-permuted space:
        #   p0: (0:512, 512:1024)      p1: (1024:1536, 1536:2048)
        #   p2: (2048:2560, 2560:3072) p3: (3072:3584, 3584:4096)
        # mm1-A rhs (q on rows 0:64), mm1-B rhs (q on rows 64:128):
        def q_rhs(p):
            if p == 0:
                return qlo[0:FIN, 0:MT], kqow[FIN:128, MT : 2 * MT]
            if p == 1:
                return qlo[0:FIN, MT : 2 * MT], kqow[FIN:128, 1536:HALF]
            return qoth[0:FIN, p - 2, :], qoth[FIN:128, p - 2, :]

        OUTCOL = {
            0: (0, MT), 1: (1024, 1536),
            2: (2048, 2560), 3: (3072, 3584),
        }

        pend = []
        po_cur = [None]

        def flush():
            p_, j_, s_ = pend.pop(0)
            if j_ == 0:
                po_cur[0] = ppo.tile([128, MT], f32, name="po", tag="po")
            po = po_cur[0]
            # ONE accumulation group for the shared bank: start only on the
            # very first matmul, stop only on the very last.
            # accumulation groups are per (partition-range x bank): each
            # column half starts/stops its own group
            nc.tensor.matmul(
                po[0:FOUT, :], vt_sb[:, j_, :], s_[:, 0:MT],
                start=(j_ == 0), stop=(j_ == N_LT - 1), skip_group_check=True,
            )
            nc.tensor.matmul(
                po[FOUT:128, :], vt_sb[:, j_, :], s_[:, MT : 2 * MT],
                start=(j_ == 0), stop=(j_ == N_LT - 1), skip_group_check=True,
            )
            if j_ == N_LT - 1:
                cA, cB = OUTCOL[p_]
                o_sb = opool.tile([128, MT], f32, name="o_sb")
                if p_ == N_P - 1:
                    # split the last drain across both engines (tail latency)
                    nc.vector.tensor_copy(o_sb[:, 0:256], po[:, 0:256])
                    nc.scalar.copy(o_sb[:, 256:MT], po[:, 256:MT])
                elif p_ % 2 == 0:
                    nc.scalar.copy(o_sb, po)
                else:
                    nc.vector.tensor_copy(o_sb, po)
                nc.sync.dma_start(out_d[:, cA : cA + MT], o_sb[0:FOUT, :])
                nc.gpsimd.dma_start(out_d[:, cB : cB + MT], o_sb[FOUT:128, :])

        for p in range(N_P):
            qa, qb = q_rhs(p)
            for j in range(N_LT):
                u = p * N_LT + j
                if u in INTERLEAVE:
                    INTERLEAVE[u]()
                jsl = slice(j * LT, (j + 1) * LT)
                ps = pps.tile([128, 2 * MT], f32, name="ps", tag="ps")
                nc.tensor.matmul(
                    ps[:, 0:MT], kqow[0:FIN, jsl], qa, start=True, stop=True
                )
                nc.tensor.matmul(
                    ps[:, MT : 2 * MT], khi[FIN:128, jsl], qb, start=True, stop=True
                )
                s_sb = spool.tile([128, 2 * MT], bf16, name="s_sb")
                if _relu_on_act(u):
                    nc.scalar.activation(s_sb, ps, AF.Relu)
                else:
                    nc.vector.tensor_scalar_max(s_sb, ps, 0.0)
                pend.append((p, j, s_sb))
                while len(pend) > 3:
                    flush()
        while pend:
            flush()

    nc.finalize()
    return nc


def _get_nc():
    if "nc" not in _NC_CACHE:
        _NC_CACHE["nc"] = _build_nc()
    return _NC_CACHE["nc"]


def make_in_maps(x, K_w, K_b, Q_w, Q_b, V_w, V_b):
    """Host-side marshalling: per-core input dicts for the SPMD kernel."""
    import ml_dtypes

    bf = ml_dtypes.bfloat16
    x = np.asarray(x, np.float32)
    # xpad col c = x col (c-1); cols 0, L+1, L+2 are zero
    xpad = np.zeros((B, FIN, L + 3), np.float32)
    xpad[:, :, 1 : L + 1] = x

    def wT(w):  # [co, ci, t] -> per-tap [ci, co]
        a = np.transpose(np.asarray(w, np.float32), (2, 1, 0))
        return a[0], a[1], a[2]

    kt0, kt1, kt2 = wT(K_w)
    qt0, qt1, qt2 = wT(Q_w)
    vt0, vt1, vt2 = (t / SQRT_KS for t in wT(V_w))
    kqw = np.zeros((128, 4 * FOUT), np.float32)
    kqw[0:FIN, 0:FOUT] = kt0
    kqw[FIN:128, 0:FOUT] = kt1
    kqw[0:FIN, FOUT : 2 * FOUT] = qt0
    kqw[FIN:128, FOUT : 2 * FOUT] = qt1
    kqw[0:FIN, 2 * FOUT : 3 * FOUT] = kt2
    kqw[0:FIN, 3 * FOUT : 4 * FOUT] = qt2
    vw = np.zeros((128, 2 * FOUT), np.float32)
    vw[0:FIN, 0:FOUT] = vt0
    vw[FIN:128, 0:FOUT] = vt1
    vw[0:FIN, FOUT : 2 * FOUT] = vt2
    kqb = np.zeros((128, 2), np.float32)
    kqb[0:FIN, 0] = np.asarray(K_b, np.float32)
    kqb[FIN:128, 0] = np.asarray(Q_b, np.float32)
    kqb[0:FIN, 1] = np.asarray(Q_b, np.float32)
    kqb[FIN:128, 1] = np.asarray(Q_b, np.float32)
    vb = np.tile((np.asarray(V_b, np.float32) / SQRT_KS), 8).reshape(1, 8 * FOUT)

    def shift_stack(a, lo):  # [64, BLK] window + 1-shifted copy
        return np.concatenate([a[:, lo : lo + BLK], a[:, lo + 1 : lo + BLK + 1]], 0)

    cast = lambda a: np.ascontiguousarray(a.astype(bf))
    in_maps = []
    for core in range(NCORES):
        b, h = divmod(core, 2)
        own, oth = h * HALF, (1 - h) * HALF
        xd = np.concatenate(
            [shift_stack(xpad[b], own), shift_stack(xpad[b], oth)], 1
        )
        in_maps.append(
            dict(xd=cast(xd), kqw=cast(kqw), vw=cast(vw), kqb=kqb, vb=vb)
        )
    return in_maps


def assemble(results):
    out = np.empty((B, FOUT, L), np.float32)
    for b in range(B):
        # core (b, h) returns columns in [own half | other half] order
        r0 = results[2 * b]["out"]          # h=0: [0:2048 | 2048:4096] natural
        r1 = results[2 * b + 1]["out"]      # h=1: [2048:4096 | 0:2048]
        out[b, :, 0:HALF] = r0[:, 0:HALF] + r1[:, HALF:L]
        out[b, :, HALF:L] = r0[:, HALF:L] + r1[:, 0:HALF]
    return out


def kernel(x, K_w, K_b, Q_w, Q_b, V_w, V_b):
    from concourse.bass_utils import run_bass_kernel_spmd

    nc = _get_nc()
    in_maps = make_in_maps(x, K_w, K_b, Q_w, Q_b, V_w, V_b)
    res = run_bass_kernel_spmd(nc, in_maps, core_ids=list(range(NCORES)))
    return assemble(res.results)


# revision 28
# speedup vs baseline: 1.0125x; 1.0125x over previous
"""Conv-QKV self-attention (CSA) Trainium2 Bass kernel, v4.

Reference computation (per batch b):
    k = conv1d(x, K_w, K_b); q = conv1d(x, Q_w, Q_b); v = conv1d(x, V_w, V_b)
    scores = relu(k^T q)                # [L, L], contraction over 64 channels
    out = v @ scores / sqrt(3)          # [64, L], contraction over L
Sharding: 8 cores = 4 batches x 2 l-halves of the score matrix.  Each core
computes a PARTIAL out (contraction over its l-half) for the full m range;
the host sums the two partials per batch.  1/sqrt(3) folded into V weights.

v4 design notes (HW-trace driven, on top of v3):
 - The flash phase is DVE+ACT bound, not PE bound: the relu copy of scores
   out of PSUM (fp32, 1 elem/lane/cycle, no 2x modes for fp32-from-PSUM on
   TRN2) costs ~1.2us per [128,1024] tile and there are 64 such tiles.
   Everything else is scheduled to hide under that ~40us/engine wall.
 - mm2 is COLUMN-PACKED: two 512-col m-tiles accumulate concurrently in the
   two column halves of the PE array (tile_position (0,0)/(0,64) via psum
   partition offsets), so PE flash work drops 42.7us -> ~29us.  The shared
   PSUM bank uses ONE accumulation group: start=True only on the very first
   matmul, stop=True only on the very last (has_written bits are
   per-element; start clears the whole bank).
 - Each flash unit = (l-tile j, m-pair): mm1 row-packed pair (k[j]@h0 ->
   cols mA, k[j]@h64 -> cols mB of one 2-bank psum tile), ONE relu op
   [128,1024] (engine chosen by a 30/34 DVE/ACT balance pattern), mm2
   col-packed pair into po[0:64]/po[64:128].
 - Conv copies merged: the fused K|Q conv writes k to psum rows 0:64 and q
   to rows 64:128; ONE [128,512] bias-copy per group (engine cost is free
   dim only).  q-other convs are col-packed pairs (2 groups concurrently on
   the two PE column halves -> one [128,512] copy per 2 groups).  vT convs
   write 8 tiles into one psum bank -> one [128,512] copy per 8 tiles.
   Row-half duplicates (k on h64, q on h0) are made by sbuf->sbuf DMA, off
   the engines.  mm1-B reads q directly from the conv output rows 64:128
   (no duplication); the other-block q pairs need no dup at all.
 - PSUM: score pool 3x[128,1024] (6 banks) + po pool 2x[128,512] (2 banks)
   = exactly 8; conv/warmup psum cycles through the same two rings.
 - preamble: tiny first x chunk + weights first on the gpsimd queue get the
   first warmup matmul ~1.5us earlier; warmup + heat keep HAM at 8/8.
"""

import numpy as np

FIN, FOUT, KS = 64, 64, 3
B, L = 4, 4096
HALF = L // 2            # per-core l range
NCORES = 8
MT = 512                 # m tile (PSUM bank free dim, fp32)
LT = 128                 # l tile (PE partition dim)
N_LT = HALF // LT        # 16 l-tiles in this core's half
N_G = HALF // MT         # 4 conv groups per 2048-col block
N_P = 4                  # m-pairs (each 2x512 cols)
BLK = HALF + 2           # 2050: x block incl +-1 halo
SQRT_KS = float(np.sqrt(KS))
N_WARM = 4               # N=384 warmup matmuls (HAM ramp + DMA preamble cover)
N_ACT_RELU = 35          # relu tiles on ACT (faster); DVE gets 64 - this

_NC_CACHE = {}


def _relu_on_act(u):
    # spread N_ACT_RELU ACT-relus evenly over the 64 units
    return (u * N_ACT_RELU) // 64 != ((u + 1) * N_ACT_RELU) // 64


def _build_nc():
    from contextlib import ExitStack

    import concourse.tile as tile
    from concourse import bacc, mybir

    f32 = mybir.dt.float32
    bf16 = mybir.dt.bfloat16
    AF = mybir.ActivationFunctionType

    nc = bacc.Bacc("TRN2", target_bir_lowering=False)

    # x as two 2050-col halo blocks [own | other], each with a 1-shifted
    # copy on partitions 64:128 (K-stacked taps 0+1), bf16.
    xd_d = nc.dram_tensor("xd", [128, 2 * BLK], bf16, kind="ExternalInput")
    # fused conv weights [128, 256]:
    #   [:,   0: 64] = [Kt0;Kt1]   [:,  64:128] = [Qt0;Qt1]
    #   [0:64,128:192] = Kt2       [0:64,192:256] = Qt2   (rows 64:128 zero)
    kqw_d = nc.dram_tensor("kqw", [128, 4 * FOUT], bf16, kind="ExternalInput")
    # v weights (1/sqrt(3) folded): [:,0:64] = [Vt0;Vt1], [0:64,64:128] = Vt2
    vw_d = nc.dram_tensor("vw", [128, 2 * FOUT], bf16, kind="ExternalInput")
    # col 0 = [K_b; Q_b]; col 1 = [Q_b; Q_b]
    kqb_d = nc.dram_tensor("kqb", [128, 2], f32, kind="ExternalInput")
    # V bias (1/sqrt(3) folded) tiled 8x: [1, 512]
    vb_d = nc.dram_tensor("vb", [1, 8 * FOUT], f32, kind="ExternalInput")
    out_d = nc.dram_tensor("out", [FOUT, L], f32, kind="ExternalOutput")

    with tile.TileContext(nc) as tc, ExitStack() as ctx:
        consts = ctx.enter_context(tc.tile_pool(name="consts", bufs=1))
        big = ctx.enter_context(tc.tile_pool(name="big", bufs=1))
        pps = ctx.enter_context(tc.tile_pool(name="pps", bufs=3, space="PSUM"))
        ppo = ctx.enter_context(tc.tile_pool(name="ppo", bufs=2, space="PSUM"))
        spool = ctx.enter_context(tc.tile_pool(name="spool", bufs=6))
        opool = ctx.enter_context(tc.tile_pool(name="opool", bufs=2))

        # ---- DMA preamble --------------------------------------------
        xd_sb = consts.tile([128, 2 * BLK], bf16)
        # tiny first chunk gates the warmup burst; then the g0 halo.
        # Four queues in parallel: sync = own x, scalar = kq consts,
        # vector = v consts, gpsimd = other-block x + later dups.
        nc.sync.dma_start(out=xd_sb[:, 0:256], in_=xd_d[:, 0:256])
        nc.sync.dma_start(out=xd_sb[:, 256:514], in_=xd_d[:, 256:514])
        kqw_sb = consts.tile([128, 4 * FOUT], bf16)
        nc.scalar.dma_start(out=kqw_sb, in_=kqw_d[:, :])
        kqb_sb = consts.tile([128, 2], f32)
        nc.scalar.dma_start(out=kqb_sb, in_=kqb_d[:, :])
        vw_sb = consts.tile([128, 2 * FOUT], bf16)
        nc.gpsimd.dma_start(out=vw_sb, in_=vw_d[:, :])
        vb_sb = consts.tile([128, 8 * FOUT], f32)
        nc.gpsimd.dma_start(out=vb_sb, in_=vb_d[:, :].to_broadcast([128, 8 * FOUT]))
        # rest of own block split across sync+scalar queues (conv groups
        # are DMA-paced), other block on gpsimd
        nc.scalar.dma_start(out=xd_sb[:, 514:1026], in_=xd_d[:, 514:1026])
        nc.sync.dma_start(out=xd_sb[:, 1026:1538], in_=xd_d[:, 1026:1538])
        nc.scalar.dma_start(out=xd_sb[:, 1538:2050], in_=xd_d[:, 1538:2050])
        nc.gpsimd.dma_start(out=xd_sb[:, BLK : BLK + 1026], in_=xd_d[:, BLK : BLK + 1026])
        nc.gpsimd.dma_start(
            out=xd_sb[:, BLK + 1026 : 2 * BLK], in_=xd_d[:, BLK + 1026 : 2 * BLK]
        )

        # conv outputs
        kqow = big.tile([128, HALF], bf16)   # rows 0:64 k+Kb, rows 64:128 q+Qb
        khi = big.tile([128, HALF], bf16)    # rows 64:128 = k+Kb (dup)
        qlo = big.tile([128, 2 * MT], bf16)  # rows 0:64: q cols {0:512,1024:1536}
        qoth = big.tile([128, 2, MT], bf16)  # [:,t,:]: rows 0:64 q grp 2t, rows 64:128 grp 2t+1
        vt_sb = big.tile([128, N_LT, FOUT], bf16)

        # ---- warmup burst on the first landed x chunk ----------------
        # HAM needs ~3.4us of dense, real-data PE activity to reach 8/8.
        # A short N=128 burst covers the kqw-DMA wait; the conv matmuls
        # and N=384 heats between conv groups provide the rest, so the
        # clocks ramp ~when the flash loop starts.
        # The HAM clock gate watches CHIP-WIDE switching activity: the
        # warmup must keep DVE and ACT toggling real data too, not just
        # the PE, or unlucky cores stay at 4/8 deep into the flash loop.
        wscr = consts.tile([128, 512], bf16)
        for i in range(8):
            wp = pps.tile([128, 2 * MT], f32, name="wp", tag="ps")
            nc.tensor.matmul(
                wp[:, 0:128], xd_sb[:, 0:128], xd_sb[:, 128:256],
                start=True, stop=True,
            )
            if i % 2 == 0:
                nc.vector.tensor_scalar_max(wscr[:, 0:256], xd_sb[:, 0:256], 0.0)
            else:
                nc.scalar.activation(wscr[:, 0:256], xd_sb[:, 0:256], AF.Relu)
        for i in range(N_WARM):
            wp = pps.tile([128, 2 * MT], f32, name="wp", tag="ps")
            nc.tensor.matmul(
                wp[:, 0:384], xd_sb[:, 0:128], xd_sb[:, 128:512],
                start=True, stop=True,
            )
            if i % 2 == 0:
                nc.vector.tensor_scalar_max(wscr, xd_sb[:, 0:512], 0.0)
            else:
                nc.scalar.activation(wscr, xd_sb[:, 0:512], AF.Relu)

        def heat(n, spin=False):
            for _ in range(n):
                hp = pps.tile([128, 2 * MT], f32, name="hp", tag="ps")
                nc.tensor.matmul(
                    hp[:, 0:384], xd_sb[:, 0:128], xd_sb[:, 128:512],
                    start=True, stop=True,
                )
            if spin:
                nc.vector.tensor_scalar_max(wscr, xd_sb[:, 0:512], 0.0)

        # ---- fused K|Q conv over the own block -----------------------
        for g in range(N_G):
            p = pps.tile([128, 2 * MT], f32, name="pkq", tag="ps")
            nc.tensor.matmul(
                p[:, 0:MT], kqw_sb[:, 0:128], xd_sb[:, g * MT : g * MT + MT],
                start=True, stop=False,
            )
            nc.tensor.matmul(
                p[:, 0:MT], kqw_sb[0:FIN, 128:256],
                xd_sb[0:FIN, g * MT + 2 : g * MT + 2 + MT],
                start=False, stop=True,
            )
            gsl = slice(g * MT, (g + 1) * MT)
            if g % 2 == 0:
                nc.scalar.activation(
                    kqow[:, gsl], p[:, 0:MT], AF.Identity, bias=kqb_sb[:, 0:1]
                )
            else:
                nc.vector.tensor_scalar_add(kqow[:, gsl], p[:, 0:MT], kqb_sb[:, 0:1])
            heat(2, spin=(g % 2 == 0))
            # khi dup per group so the first flash units unblock early
            nc.sync.dma_start(out=khi[FIN:128, gsl], in_=kqow[0:FIN, gsl])
            if g == 0:
                nc.gpsimd.dma_start(out=qlo[0:FIN, 0:MT], in_=kqow[FIN:128, 0:MT])
            if g == 2:
                nc.gpsimd.dma_start(
                    out=qlo[0:FIN, MT : 2 * MT], in_=kqow[FIN:128, 1024:1536]
                )

        # ---- conv work interleaved into the early flash stream -------
        # vT tiles: 8 per psum bank (one engine copy per bank); q-other
        # col-packed conv pairs.  Emitted between early flash units so PE
        # keeps the engines fed while these fill in; their consumers
        # (mm2 / pairs 2-3) are many units downstream.
        pv_cur = [None]
        pq_cur = [None]

        def vt_block(blk):
            pv = ppo.tile([128, MT], f32, name="pv", tag="po")
            pv_cur[0] = pv
            for i in range(8):
                j = blk * 8 + i
                vsl = slice(i * FOUT, (i + 1) * FOUT)
                nc.tensor.matmul(
                    pv[:, vsl], xd_sb[:, j * LT : j * LT + LT], vw_sb[:, 0:FOUT],
                    start=(i == 0), stop=False,
                )
                nc.tensor.matmul(
                    pv[:, vsl], xd_sb[0:FIN, j * LT + 2 : j * LT + 2 + LT],
                    vw_sb[0:FIN, FOUT:128],
                    start=False, stop=(i == 7),
                )
                if i % 2 == 1:
                    # N=64 streams read as "idle" to the HAM activity
                    # monitor; keep real N=384 bursts in the mix
                    heat(1)
            nc.vector.tensor_add(
                vt_sb[:, blk * 8 : (blk + 1) * 8, :], pv, vb_sb[:, 0:MT]
            )

        def qoth_conv(t):
            pq = pps.tile([128, 2 * MT], f32, name="pq", tag="ps")
            pq_cur[0] = pq
            for tap in range(2):
                for h in range(2):
                    lo = BLK + (2 * t + h) * MT + 2 * tap
                    osl = pq[h * FOUT : (h + 1) * FOUT, 0:MT]
                    if tap == 0:
                        nc.tensor.matmul(
                            osl, kqw_sb[:, 64:128], xd_sb[:, lo : lo + MT],
                            start=True, stop=False, skip_group_check=True,
                        )
                    else:
                        nc.tensor.matmul(
                            osl, kqw_sb[0:FIN, 192:256], xd_sb[0:FIN, lo : lo + MT],
                            start=False, stop=True, skip_group_check=True,
                        )

        def qoth_copy(t):
            pq = pq_cur[0]
            if t == 0:
                nc.scalar.activation(
                    qoth[:, t, :], pq[:, 0:MT], AF.Identity, bias=kqb_sb[:, 1:2]
                )
            else:
                nc.vector.tensor_scalar_add(qoth[:, t, :], pq[:, 0:MT], kqb_sb[:, 1:2])

        INTERLEAVE = {
            1: lambda: vt_block(0),
            3: lambda: vt_block(1),
            6: lambda: qoth_conv(0),
            7: lambda: qoth_copy(0),
            8: lambda: qoth_conv(1),
            9: lambda: qoth_copy(1),
        }

        # ---- flash loop ----------------------------------------------
        # m-pair p covers out columns (pA, pB) in the block-permuted space:
        #   p0: (0:512, 512:1024)      p1: (1024:1536, 1536:2048)
        #   p2: (2048:2560, 2560:3072) p3: (3072:3584, 3584:4096)
        # mm1-A rhs (q on rows 0:64), mm1-B rhs (q on rows 64:128):
        def q_rhs(p):
            if p == 0:
                return qlo[0:FIN, 0:MT], kqow[FIN:128, MT : 2 * MT]
            if p == 1:
                return qlo[0:FIN, MT : 2 * MT], kqow[FIN:128, 1536:HALF]
            return qoth[0:FIN, p - 2, :], qoth[FIN:128, p - 2, :]

        OUTCOL = {
            0: (0, MT), 1: (1024, 1536),
            2: (2048, 2560), 3: (3072, 3584),
        }

        pend = []
        po_cur = [None]

        def flush():
            p_, j_, s_ = pend.pop(0)
            if j_ == 0:
                po_cur[0] = ppo.tile([128, MT], f32, name="po", tag="po")
            po = po_cur[0]
            # ONE accumulation group for the shared bank: start only on the
            # very first matmul, stop only on the very last.
            # accumulation groups are per (partition-range x bank): each
            # column half starts/stops its own group
            nc.tensor.matmul(
                po[0:FOUT, :], vt_sb[:, j_, :], s_[:, 0:MT],
                start=(j_ == 0), stop=(j_ == N_LT - 1), skip_group_check=True,
            )
            nc.tensor.matmul(
                po[FOUT:128, :], vt_sb[:, j_, :], s_[:, MT : 2 * MT],
                start=(j_ == 0), stop=(j_ == N_LT - 1), skip_group_check=True,
            )
            if j_ == N_LT - 1:
                cA, cB = OUTCOL[p_]
                o_sb = opool.tile([128, MT], f32, name="o_sb")
                if p_ == N_P - 1:
                    # split the last drain across both engines (tail latency)
                    nc.vector.tensor_copy(o_sb[:, 0:256], po[:, 0:256])
                    nc.scalar.copy(o_sb[:, 256:MT], po[:, 256:MT])
                elif p_ % 2 == 0:
                    nc.scalar.copy(o_sb, po)
                else:
                    nc.vector.tensor_copy(o_sb, po)
                nc.sync.dma_start(out_d[:, cA : cA + MT], o_sb[0:FOUT, :])
                nc.gpsimd.dma_start(out_d[:, cB : cB + MT], o_sb[FOUT:128, :])

        for p in range(N_P):
            qa, qb = q_rhs(p)
            for j in range(N_LT):
                u = p * N_LT + j
                if u in INTERLEAVE:
                    INTERLEAVE[u]()
                jsl = slice(j * LT, (j + 1) * LT)
                ps = pps.tile([128, 2 * MT], f32, name="ps", tag="ps")
                nc.tensor.matmul(
                    ps[:, 0:MT], kqow[0:FIN, jsl], qa, start=True, stop=True
                )
                nc.tensor.matmul(
                    ps[:, MT : 2 * MT], khi[FIN:128, jsl], qb, start=True, stop=True
                )
                s_sb = spool.tile([128, 2 * MT], bf16, name="s_sb")
                if _relu_on_act(u):
                    nc.scalar.activation(s_sb, ps, AF.Relu)
                else:
                    nc.vector.tensor_scalar_max(s_sb, ps, 0.0)
                pend.append((p, j, s_sb))
                while len(pend) > 3:
                    flush()
        while pend:
            flush()

    nc.finalize()
    return nc


def _get_nc():
    if "nc" not in _NC_CACHE:
        _NC_CACHE["nc"] = _build_nc()
    return _NC_CACHE["nc"]


def make_in_maps(x, K_w, K_b, Q_w, Q_b, V_w, V_b):
    """Host-side marshalling: per-core input dicts for the SPMD kernel."""
    import ml_dtypes

    bf = ml_dtypes.bfloat16
    x = np.asarray(x, np.float32)
    # xpad col c = x col (c-1); cols 0, L+1, L+2 are zero
    xpad = np.zeros((B, FIN, L + 3), np.float32)
    xpad[:, :, 1 : L + 1] = x

    def wT(w):  # [co, ci, t] -> per-tap [ci, co]
        a = np.transpose(np.asarray(w, np.float32), (2, 1, 0))
        return a[0], a[1], a[2]

    kt0, kt1, kt2 = wT(K_w)
    qt0, qt1, qt2 = wT(Q_w)
    vt0, vt1, vt2 = (t / SQRT_KS for t in wT(V_w))
    kqw = np.zeros((128, 4 * FOUT), np.float32)
    kqw[0:FIN, 0:FOUT] = kt0
    kqw[FIN:128, 0:FOUT] = kt1
    kqw[0:FIN, FOUT : 2 * FOUT] = qt0
    kqw[FIN:128, FOUT : 2 * FOUT] = qt1
    kqw[0:FIN, 2 * FOUT : 3 * FOUT] = kt2
    kqw[0:FIN, 3 * FOUT : 4 * FOUT] = qt2
    vw = np.zeros((128, 2 * FOUT), np.float32)
    vw[0:FIN, 0:FOUT] = vt0
    vw[FIN:128, 0:FOUT] = vt1
    vw[0:FIN, FOUT : 2 * FOUT] = vt2
    kqb = np.zeros((128, 2), np.float32)
    kqb[0:FIN, 0] = np.asarray(K_b, np.float32)
    kqb[FIN:128, 0] = np.asarray(Q_b, np.float32)
    kqb[0:FIN, 1] = np.asarray(Q_b, np.float32)
    kqb[FIN:128, 1] = np.asarray(Q_b, np.float32)
    vb = np.tile((np.asarray(V_b, np.float32) / SQRT_KS), 8).reshape(1, 8 * FOUT)

    def shift_stack(a, lo):  # [64, BLK] window + 1-shifted copy
        return np.concatenate([a[:, lo : lo + BLK], a[:, lo + 1 : lo + BLK + 1]], 0)

    cast = lambda a: np.ascontiguousarray(a.astype(bf))
    in_maps = []
    for core in range(NCORES):
        b, h = divmod(core, 2)
        own, oth = h * HALF, (1 - h) * HALF
        xd = np.concatenate(
            [shift_stack(xpad[b], own), shift_stack(xpad[b], oth)], 1
        )
        in_maps.append(
            dict(xd=cast(xd), kqw=cast(kqw), vw=cast(vw), kqb=kqb, vb=vb)
        )
    return in_maps


def assemble(results):
    out = np.empty((B, FOUT, L), np.float32)
    for b in range(B):
        # core (b, h) returns columns in [own half | other half] order
        r0 = results[2 * b]["out"]          # h=0: [0:2048 | 2048:4096] natural
        r1 = results[2 * b + 1]["out"]      # h=1: [2048:4096 | 0:2048]
        out[b, :, 0:HALF] = r0[:, 0:HALF] + r1[:, HALF:L]
        out[b, :, HALF:L] = r0[:, HALF:L] + r1[:, 0:HALF]
    return out


def kernel(x, K_w, K_b, Q_w, Q_b, V_w, V_b):
    from concourse.bass_utils import run_bass_kernel_spmd

    nc = _get_nc()
    in_maps = make_in_maps(x, K_w, K_b, Q_w, Q_b, V_w, V_b)
    res = run_bass_kernel_spmd(nc, in_maps, core_ids=list(range(NCORES)))
    return assemble(res.results)


# revision 30
# speedup vs baseline: 1.0544x; 1.0414x over previous
"""Conv-QKV self-attention (CSA) Trainium2 Bass kernel, v4.

Reference computation (per batch b):
    k = conv1d(x, K_w, K_b); q = conv1d(x, Q_w, Q_b); v = conv1d(x, V_w, V_b)
    scores = relu(k^T q)                # [L, L], contraction over 64 channels
    out = v @ scores / sqrt(3)          # [64, L], contraction over L
Sharding: 8 cores = 4 batches x 2 l-halves of the score matrix.  Each core
computes a PARTIAL out (contraction over its l-half) for the full m range;
the host sums the two partials per batch.  1/sqrt(3) folded into V weights.

v4 design notes (HW-trace driven, on top of v3):
 - The flash phase is DVE+ACT bound, not PE bound: the relu copy of scores
   out of PSUM (fp32, 1 elem/lane/cycle, no 2x modes for fp32-from-PSUM on
   TRN2) costs ~1.2us per [128,1024] tile and there are 64 such tiles.
   Everything else is scheduled to hide under that ~40us/engine wall.
 - mm2 is COLUMN-PACKED: two 512-col m-tiles accumulate concurrently in the
   two column halves of the PE array (tile_position (0,0)/(0,64) via psum
   partition offsets), so PE flash work drops 42.7us -> ~29us.  The shared
   PSUM bank uses ONE accumulation group: start=True only on the very first
   matmul, stop=True only on the very last (has_written bits are
   per-element; start clears the whole bank).
 - Each flash unit = (l-tile j, m-pair): mm1 row-packed pair (k[j]@h0 ->
   cols mA, k[j]@h64 -> cols mB of one 2-bank psum tile), ONE relu op
   [128,1024] (engine chosen by a 30/34 DVE/ACT balance pattern), mm2
   col-packed pair into po[0:64]/po[64:128].
 - Conv copies merged: the fused K|Q conv writes k to psum rows 0:64 and q
   to rows 64:128; ONE [128,512] bias-copy per group (engine cost is free
   dim only).  q-other convs are col-packed pairs (2 groups concurrently on
   the two PE column halves -> one [128,512] copy per 2 groups).  vT convs
   write 8 tiles into one psum bank -> one [128,512] copy per 8 tiles.
   Row-half duplicates (k on h64, q on h0) are made by sbuf->sbuf DMA, off
   the engines.  mm1-B reads q directly from the conv output rows 64:128
   (no duplication); the other-block q pairs need no dup at all.
 - PSUM: score pool 3x[128,1024] (6 banks) + po pool 2x[128,512] (2 banks)
   = exactly 8; conv/warmup psum cycles through the same two rings.
 - preamble: tiny first x chunk + weights first on the gpsimd queue get the
   first warmup matmul ~1.5us earlier; warmup + heat keep HAM at 8/8.
"""

import numpy as np

FIN, FOUT, KS = 64, 64, 3
B, L = 4, 4096
HALF = L // 2            # per-core l range
NCORES = 8
MT = 512                 # m tile (PSUM bank free dim, fp32)
LT = 128                 # l tile (PE partition dim)
N_LT = HALF // LT        # 16 l-tiles in this core's half
N_G = HALF // MT         # 4 conv groups per 2048-col block
N_P = 4                  # m-pairs (each 2x512 cols)
BLK = HALF + 2           # 2050: x block incl +-1 halo
SQRT_KS = float(np.sqrt(KS))
N_WARM = 4               # N=384 warmup matmuls (HAM ramp + DMA preamble cover)
N_ACT_RELU = 35          # relu tiles on ACT (faster); DVE gets 64 - this

_NC_CACHE = {}


def _relu_on_act(u):
    # spread N_ACT_RELU ACT-relus evenly over the 64 units
    return (u * N_ACT_RELU) // 64 != ((u + 1) * N_ACT_RELU) // 64


def _build_nc():
    from contextlib import ExitStack

    import concourse.tile as tile
    from concourse import bacc, mybir

    f32 = mybir.dt.float32
    bf16 = mybir.dt.bfloat16
    AF = mybir.ActivationFunctionType

    nc = bacc.Bacc("TRN2", target_bir_lowering=False)

    # x as two 2050-col halo blocks [own | other], each with a 1-shifted
    # copy on partitions 64:128 (K-stacked taps 0+1), bf16.
    xd_d = nc.dram_tensor("xd", [128, 2 * BLK], bf16, kind="ExternalInput")
    # fused conv weights [128, 256]:
    #   [:,   0: 64] = [Kt0;Kt1]   [:,  64:128] = [Qt0;Qt1]
    #   [0:64,128:192] = Kt2       [0:64,192:256] = Qt2   (rows 64:128 zero)
    kqw_d = nc.dram_tensor("kqw", [128, 4 * FOUT], bf16, kind="ExternalInput")
    # v weights (1/sqrt(3) folded): [:,0:64] = [Vt0;Vt1], [0:64,64:128] = Vt2
    vw_d = nc.dram_tensor("vw", [128, 2 * FOUT], bf16, kind="ExternalInput")
    # col 0 = [K_b; Q_b]; col 1 = [Q_b; Q_b]
    kqb_d = nc.dram_tensor("kqb", [128, 2], f32, kind="ExternalInput")
    # V bias (1/sqrt(3) folded) tiled 8x: [1, 512]
    vb_d = nc.dram_tensor("vb", [1, 8 * FOUT], f32, kind="ExternalInput")
    out_d = nc.dram_tensor("out", [FOUT, L], f32, kind="ExternalOutput")

    with tile.TileContext(nc) as tc, ExitStack() as ctx:
        consts = ctx.enter_context(tc.tile_pool(name="consts", bufs=1))
        big = ctx.enter_context(tc.tile_pool(name="big", bufs=1))
        pps = ctx.enter_context(tc.tile_pool(name="pps", bufs=3, space="PSUM"))
        ppo = ctx.enter_context(tc.tile_pool(name="ppo", bufs=2, space="PSUM"))
        spool = ctx.enter_context(tc.tile_pool(name="spool", bufs=6))
        opool = ctx.enter_context(tc.tile_pool(name="opool", bufs=2))

        # ---- DMA preamble --------------------------------------------
        xd_sb = consts.tile([128, 2 * BLK], bf16)
        # tiny first chunk gates the warmup burst; then the g0 halo.
        # Four queues in parallel: sync = own x, scalar = kq consts,
        # vector = v consts, gpsimd = other-block x + later dups.
        nc.sync.dma_start(out=xd_sb[:, 0:256], in_=xd_d[:, 0:256])
        nc.sync.dma_start(out=xd_sb[:, 256:514], in_=xd_d[:, 256:514])
        kqw_sb = consts.tile([128, 4 * FOUT], bf16)
        nc.scalar.dma_start(out=kqw_sb, in_=kqw_d[:, :])
        kqb_sb = consts.tile([128, 2], f32)
        nc.scalar.dma_start(out=kqb_sb, in_=kqb_d[:, :])
        vw_sb = consts.tile([128, 2 * FOUT], bf16)
        nc.gpsimd.dma_start(out=vw_sb, in_=vw_d[:, :])
        vb_sb = consts.tile([128, 8 * FOUT], f32)
        nc.gpsimd.dma_start(out=vb_sb, in_=vb_d[:, :].to_broadcast([128, 8 * FOUT]))
        # rest of own block split across sync+scalar queues (conv groups
        # are DMA-paced), other block on gpsimd
        nc.scalar.dma_start(out=xd_sb[:, 514:1026], in_=xd_d[:, 514:1026])
        nc.sync.dma_start(out=xd_sb[:, 1026:1538], in_=xd_d[:, 1026:1538])
        nc.scalar.dma_start(out=xd_sb[:, 1538:2050], in_=xd_d[:, 1538:2050])
        nc.gpsimd.dma_start(out=xd_sb[:, BLK : BLK + 1026], in_=xd_d[:, BLK : BLK + 1026])
        nc.gpsimd.dma_start(
            out=xd_sb[:, BLK + 1026 : 2 * BLK], in_=xd_d[:, BLK + 1026 : 2 * BLK]
        )

        # conv outputs
        kqow = big.tile([128, HALF], bf16)   # rows 0:64 k+Kb, rows 64:128 q+Qb
        khi = big.tile([128, HALF], bf16)    # rows 64:128 = k+Kb (dup)
        qlo = big.tile([128, 2 * MT], bf16)  # rows 0:64: q cols {0:512,1024:1536}
        qoth = big.tile([128, 2, MT], bf16)  # [:,t,:]: rows 0:64 q grp 2t, rows 64:128 grp 2t+1
        vt_sb = big.tile([128, N_LT, FOUT], bf16)

        # ---- warmup burst on the first landed x chunk ----------------
        # HAM needs ~3.4us of dense, real-data PE activity to reach 8/8.
        # A short N=128 burst covers the kqw-DMA wait; the conv matmuls
        # and N=384 heats between conv groups provide the rest, so the
        # clocks ramp ~when the flash loop starts.
        # The HAM clock gate watches CHIP-WIDE switching activity: the
        # warmup must keep DVE and ACT toggling real data too, not just
        # the PE, or unlucky cores stay at 4/8 deep into the flash loop.
        wscr = consts.tile([128, 512], bf16)
        for i in range(8):
            wp = pps.tile([128, 2 * MT], f32, name="wp", tag="ps")
            nc.tensor.matmul(
                wp[:, 0:128], xd_sb[:, 0:128], xd_sb[:, 128:256],
                start=True, stop=True,
            )
            if i % 2 == 0:
                nc.vector.tensor_scalar_max(wscr[:, 0:256], xd_sb[:, 0:256], 0.0)
            else:
                nc.scalar.activation(wscr[:, 0:256], xd_sb[:, 0:256], AF.Relu)
        for i in range(10):
            wp = pps.tile([128, 2 * MT], f32, name="wp", tag="ps")
            nc.tensor.matmul(
                wp[:, 0:384], xd_sb[:, 0:128], xd_sb[:, 128:512],
                start=True, stop=True,
            )
            if i % 2 == 0:
                nc.vector.tensor_scalar_max(wscr, xd_sb[:, 0:512], 0.0)
            else:
                nc.scalar.activation(wscr, xd_sb[:, 0:512], AF.Relu)

        def heat(n, spin=False):
            for _ in range(n):
                hp = pps.tile([128, 2 * MT], f32, name="hp", tag="ps")
                nc.tensor.matmul(
                    hp[:, 0:384], xd_sb[:, 0:128], xd_sb[:, 128:512],
                    start=True, stop=True,
                )
            if spin:
                nc.vector.tensor_scalar_max(wscr, xd_sb[:, 0:512], 0.0)

        # ---- fused K|Q conv over the own block -----------------------
        for g in range(N_G):
            p = pps.tile([128, 2 * MT], f32, name="pkq", tag="ps")
            nc.tensor.matmul(
                p[:, 0:MT], kqw_sb[:, 0:128], xd_sb[:, g * MT : g * MT + MT],
                start=True, stop=False,
            )
            nc.tensor.matmul(
                p[:, 0:MT], kqw_sb[0:FIN, 128:256],
                xd_sb[0:FIN, g * MT + 2 : g * MT + 2 + MT],
                start=False, stop=True,
            )
            gsl = slice(g * MT, (g + 1) * MT)
            if g % 2 == 0:
                nc.scalar.activation(
                    kqow[:, gsl], p[:, 0:MT], AF.Identity, bias=kqb_sb[:, 0:1]
                )
            else:
                nc.vector.tensor_scalar_add(kqow[:, gsl], p[:, 0:MT], kqb_sb[:, 0:1])
            heat(2, spin=(g % 2 == 0))
            # khi dup per group so the first flash units unblock early
            nc.sync.dma_start(out=khi[FIN:128, gsl], in_=kqow[0:FIN, gsl])
            if g == 0:
                nc.gpsimd.dma_start(out=qlo[0:FIN, 0:MT], in_=kqow[FIN:128, 0:MT])
            if g == 2:
                nc.gpsimd.dma_start(
                    out=qlo[0:FIN, MT : 2 * MT], in_=kqow[FIN:128, 1024:1536]
                )

        # ---- conv work interleaved into the early flash stream -------
        # vT tiles: 8 per psum bank (one engine copy per bank); q-other
        # col-packed conv pairs.  Emitted between early flash units so PE
        # keeps the engines fed while these fill in; their consumers
        # (mm2 / pairs 2-3) are many units downstream.
        pv_cur = [None]
        pq_cur = [None]

        def vt_block(blk):
            pv = ppo.tile([128, MT], f32, name="pv", tag="po")
            pv_cur[0] = pv
            for i in range(8):
                j = blk * 8 + i
                vsl = slice(i * FOUT, (i + 1) * FOUT)
                nc.tensor.matmul(
                    pv[:, vsl], xd_sb[:, j * LT : j * LT + LT], vw_sb[:, 0:FOUT],
                    start=(i == 0), stop=False,
                )
                nc.tensor.matmul(
                    pv[:, vsl], xd_sb[0:FIN, j * LT + 2 : j * LT + 2 + LT],
                    vw_sb[0:FIN, FOUT:128],
                    start=False, stop=(i == 7),
                )
                if i % 2 == 1:
                    # N=64 streams read as "idle" to the HAM activity
                    # monitor; keep real N=384 bursts in the mix
                    heat(1, spin=(i == 3))
            nc.vector.tensor_add(
                vt_sb[:, blk * 8 : (blk + 1) * 8, :], pv, vb_sb[:, 0:MT]
            )

        def qoth_conv(t):
            pq = pps.tile([128, 2 * MT], f32, name="pq", tag="ps")
            pq_cur[0] = pq
            for tap in range(2):
                for h in range(2):
                    lo = BLK + (2 * t + h) * MT + 2 * tap
                    osl = pq[h * FOUT : (h + 1) * FOUT, 0:MT]
                    if tap == 0:
                        nc.tensor.matmul(
                            osl, kqw_sb[:, 64:128], xd_sb[:, lo : lo + MT],
                            start=True, stop=False, skip_group_check=True,
                        )
                    else:
                        nc.tensor.matmul(
                            osl, kqw_sb[0:FIN, 192:256], xd_sb[0:FIN, lo : lo + MT],
                            start=False, stop=True, skip_group_check=True,
                        )

        def qoth_copy(t):
            pq = pq_cur[0]
            if t == 0:
                nc.scalar.activation(
                    qoth[:, t, :], pq[:, 0:MT], AF.Identity, bias=kqb_sb[:, 1:2]
                )
            else:
                nc.vector.tensor_scalar_add(qoth[:, t, :], pq[:, 0:MT], kqb_sb[:, 1:2])

        INTERLEAVE = {
            1: lambda: vt_block(0),
            3: lambda: vt_block(1),
            6: lambda: qoth_conv(0),
            7: lambda: qoth_copy(0),
            8: lambda: qoth_conv(1),
            9: lambda: qoth_copy(1),
        }

        # ---- flash loop ----------------------------------------------
        # m-pair p covers out columns (pA, pB) in the block-permuted space:
        #   p0: (0:512, 512:1024)      p1: (1024:1536, 1536:2048)
        #   p2: (2048:2560, 2560:3072) p3: (3072:3584, 3584:4096)
        # mm1-A rhs (q on rows 0:64), mm1-B rhs (q on rows 64:128):
        def q_rhs(p):
            if p == 0:
                return qlo[0:FIN, 0:MT], kqow[FIN:128, MT : 2 * MT]
            if p == 1:
                return qlo[0:FIN, MT : 2 * MT], kqow[FIN:128, 1536:HALF]
            return qoth[0:FIN, p - 2, :], qoth[FIN:128, p - 2, :]

        OUTCOL = {
            0: (0, MT), 1: (1024, 1536),
            2: (2048, 2560), 3: (3072, 3584),
        }

        pend = []
        po_cur = [None]

        def flush():
            p_, j_, s_ = pend.pop(0)
            if j_ == 0:
                po_cur[0] = ppo.tile([128, MT], f32, name="po", tag="po")
            po = po_cur[0]
            # ONE accumulation group for the shared bank: start only on the
            # very first matmul, stop only on the very last.
            # accumulation groups are per (partition-range x bank): each
            # column half starts/stops its own group
            nc.tensor.matmul(
                po[0:FOUT, :], vt_sb[:, j_, :], s_[:, 0:MT],
                start=(j_ == 0), stop=(j_ == N_LT - 1), skip_group_check=True,
            )
            nc.tensor.matmul(
                po[FOUT:128, :], vt_sb[:, j_, :], s_[:, MT : 2 * MT],
                start=(j_ == 0), stop=(j_ == N_LT - 1), skip_group_check=True,
            )
            if j_ == N_LT - 1:
                cA, cB = OUTCOL[p_]
                o_sb = opool.tile([128, MT], f32, name="o_sb")
                if p_ == N_P - 1:
                    # split the last drain across both engines (tail latency)
                    nc.vector.tensor_copy(o_sb[:, 0:256], po[:, 0:256])
                    nc.scalar.copy(o_sb[:, 256:MT], po[:, 256:MT])
                elif p_ % 2 == 0:
                    nc.scalar.copy(o_sb, po)
                else:
                    nc.vector.tensor_copy(o_sb, po)
                nc.sync.dma_start(out_d[:, cA : cA + MT], o_sb[0:FOUT, :])
                nc.gpsimd.dma_start(out_d[:, cB : cB + MT], o_sb[FOUT:128, :])

        for p in range(N_P):
            qa, qb = q_rhs(p)
            for j in range(N_LT):
                u = p * N_LT + j
                if u in INTERLEAVE:
                    INTERLEAVE[u]()
                jsl = slice(j * LT, (j + 1) * LT)
                ps = pps.tile([128, 2 * MT], f32, name="ps", tag="ps")
                nc.tensor.matmul(
                    ps[:, 0:MT], kqow[0:FIN, jsl], qa, start=True, stop=True
                )
                nc.tensor.matmul(
                    ps[:, MT : 2 * MT], khi[FIN:128, jsl], qb, start=True, stop=True
                )
                s_sb = spool.tile([128, 2 * MT], bf16, name="s_sb")
                if _relu_on_act(u):
                    nc.scalar.activation(s_sb, ps, AF.Relu)
                else:
                    nc.vector.tensor_scalar_max(s_sb, ps, 0.0)
                pend.append((p, j, s_sb))
                while len(pend) > 3:
                    flush()
        while pend:
            flush()

    nc.finalize()
    return nc


def _get_nc():
    if "nc" not in _NC_CACHE:
        _NC_CACHE["nc"] = _build_nc()
    return _NC_CACHE["nc"]


def make_in_maps(x, K_w, K_b, Q_w, Q_b, V_w, V_b):
    """Host-side marshalling: per-core input dicts for the SPMD kernel."""
    import ml_dtypes

    bf = ml_dtypes.bfloat16
    x = np.asarray(x, np.float32)
    # xpad col c = x col (c-1); cols 0, L+1, L+2 are zero
    xpad = np.zeros((B, FIN, L + 3), np.float32)
    xpad[:, :, 1 : L + 1] = x

    def wT(w):  # [co, ci, t] -> per-tap [ci, co]
        a = np.transpose(np.asarray(w, np.float32), (2, 1, 0))
        return a[0], a[1], a[2]

    kt0, kt1, kt2 = wT(K_w)
    qt0, qt1, qt2 = wT(Q_w)
    vt0, vt1, vt2 = (t / SQRT_KS for t in wT(V_w))
    kqw = np.zeros((128, 4 * FOUT), np.float32)
    kqw[0:FIN, 0:FOUT] = kt0
    kqw[FIN:128, 0:FOUT] = kt1
    kqw[0:FIN, FOUT : 2 * FOUT] = qt0
    kqw[FIN:128, FOUT : 2 * FOUT] = qt1
    kqw[0:FIN, 2 * FOUT : 3 * FOUT] = kt2
    kqw[0:FIN, 3 * FOUT : 4 * FOUT] = qt2
    vw = np.zeros((128, 2 * FOUT), np.float32)
    vw[0:FIN, 0:FOUT] = vt0
    vw[FIN:128, 0:FOUT] = vt1
    vw[0:FIN, FOUT : 2 * FOUT] = vt2
    kqb = np.zeros((128, 2), np.float32)
    kqb[0:FIN, 0] = np.asarray(K_b, np.float32)
    kqb[FIN:128, 0] = np.asarray(Q_b, np.float32)
    kqb[0:FIN, 1] = np.asarray(Q_b, np.float32)
    kqb[FIN:128, 1] = np.asarray(Q_b, np.float32)
    vb = np.tile((np.asarray(V_b, np.float32) / SQRT_KS), 8).reshape(1, 8 * FOUT)

    def shift_stack(a, lo):  # [64, BLK] window + 1-shifted copy
        return np.concatenate([a[:, lo : lo + BLK], a[:, lo + 1 : lo + BLK + 1]], 0)

    cast = lambda a: np.ascontiguousarray(a.astype(bf))
    in_maps = []
    for core in range(NCORES):
        b, h = divmod(core, 2)
        own, oth = h * HALF, (1 - h) * HALF
        xd = np.concatenate(
            [shift_stack(xpad[b], own), shift_stack(xpad[b], oth)], 1
        )
        in_maps.append(
            dict(xd=cast(xd), kqw=cast(kqw), vw=cast(vw), kqb=kqb, vb=vb)
        )
    return in_maps


def assemble(results):
    out = np.empty((B, FOUT, L), np.float32)
    for b in range(B):
        # core (b, h) returns columns in [own half | other half] order
        r0 = results[2 * b]["out"]          # h=0: [0:2048 | 2048:4096] natural
        r1 = results[2 * b + 1]["out"]      # h=1: [2048:4096 | 0:2048]
        out[b, :, 0:HALF] = r0[:, 0:HALF] + r1[:, HALF:L]
        out[b, :, HALF:L] = r0[:, HALF:L] + r1[:, 0:HALF]
    return out


def kernel(x, K_w, K_b, Q_w, Q_b, V_w, V_b):
    from concourse.bass_utils import run_bass_kernel_spmd

    nc = _get_nc()
    in_maps = make_in_maps(x, K_w, K_b, Q_w, Q_b, V_w, V_b)
    res = run_bass_kernel_spmd(nc, in_maps, core_ids=list(range(NCORES)))
    return assemble(res.results)
